# revision 1
# baseline (speedup 1.0000x reference)
"""Trainium2 Bass kernel for nn_AutoRegressiveDistribution (MADE sampling).

Self-contained: hardcodes shapes/sharding. Shards batch B across 8 cores,
runs the D-step autoregressive sampling loop fully on-device per core.

Per-core structure (rows = S*BS = 512, processed as TWO independent
half-chains of an s-pair each so the serial per-step dependency chains
overlap across engines):
  - Hidden units are permuted host-side, sorted by MADE degree (1..63),
    each degree block placed at a 32-aligned slot of a padded unit space
    (matmul operands must start at partition 0/32/64). At step i only the
    block of degree i gets a fresh pre-activation (depends on z_{<i}),
    is relu'd (to bf16), and contributes into a persistent PSUM
    accumulator OUT (batch on partitions, (s, outcol) on free).
  - z-history kept in batch-on-partitions (cheap vector ops) and
    degree-on-partitions zT (matmul rhs), bridged per step by full PE
    re-transposes (float32r) + a 32-row-group PSUM->SBUF copy.
  - The z-path matmuls use float32r (fp32 bits, full-rate for N>=256);
    the output contributions use bf16.
  - ctx_h = Wc @ ctx + b1 is precomputed once; bout is seeded into OUT
    once via ones-outer-product matmuls, so the per-step chain is
    hist-MM -> relu -> contrib -> exp -> ln -> mul/add -> transpose ->
    copy with no bias work.
"""

import numpy as np
from contextlib import ExitStack

import concourse.bass as bass
import concourse.tile as tile
from concourse import bacc, mybir
from concourse.bass_utils import run_bass_kernel_spmd

D, H, CTX, B, S = 64, 1024, 256, 1024, 4
NCORES = 8
BS = B // NCORES          # 128 batch rows per core
R = S * BS                # 512 rows per core
RH = R // 2               # rows per half-chain (s-pair)
FP32 = mybir.dt.float32
BF16 = mybir.dt.bfloat16
F32R = mybir.dt.float32r

HP = 2048  # padded hidden units: degree block i at [32*(i-1), 32*(i-1)+cnt[i])


def _made_struct():
    mh = (np.arange(H) % (D - 1)) + 1            # degrees 1..63
    perm = np.argsort(mh, kind="stable")
    mh_s = mh[perm]
    cnt = np.bincount(mh_s, minlength=D)          # cnt[d] = #units of degree d
    off = np.concatenate([[0], np.cumsum(cnt)[:-1]]).astype(np.int64)
    return mh, perm, mh_s, cnt, off


def _prep_weights(W1, b1, Wc, Wout):
    """Mask + permute + 32-pad weights host-side (cheap, O(weight size))."""
    mh, perm, mh_s, cnt, off = _made_struct()
    m0 = np.arange(1, D + 1)
    M1 = (mh[:, None] >= m0[None, :]).astype(np.float32)          # (H, D)
    mout = np.concatenate([m0, m0])                                # (2D,)
    Mout = (mout[:, None] > mh[None, :]).astype(np.float32)        # (2D, H)
    W1m = (W1 * M1)[perm]                   # (H, D) permuted rows
    Woutm = (Wout * Mout)[:, perm]          # (2D, H) permuted cols
    src = np.arange(H)
    pdst = 32 * (mh_s - 1) + (src - off[mh_s])   # padded slot of sorted unit
    import ml_dtypes
    bf = ml_dtypes.bfloat16
    W1T = np.zeros((D, HP), np.float32)
    W1T[:, pdst] = W1m.T
    WcT = np.zeros((CTX, HP), np.float32)
    WcT[:, pdst] = Wc[perm].T
    b1p = np.zeros((HP, 1), np.float32)
    b1p[pdst, 0] = b1[perm]
    WoutB = np.zeros((32, D - 1, 2 * D), np.float32)  # (slot, block, outcol)
    WoutB[pdst % 32, (mh_s - 1)] = Woutm[:, :].T[src]
    return W1T, WoutB, WcT, b1p


_PROGRAM_CACHE = None


def _pin_act_table():
    """Make Exp/Ln/Relu resolvable only via natural_log_exp_and_others so
    the act-table chooser doesn't thrash between the exp and ln tables
    (each LoadActFuncSet costs ~1.3us). Table positions are preserved so
    act_func_set_id stays consistent with act_info.json."""
    import concourse.bacc as bacc_mod
    from concourse import hw_specs
    orig = hw_specs.get_activation_tables
    AF = mybir.ActivationFunctionType
    pin = {AF.Exp, AF.Ln, AF.Relu}

    def filtered(arch):
        out = {}
        for name, fns in orig(arch).items():
            if name == "natural_log_exp_and_others":
                out[name] = set(fns)
            else:
                out[name] = set(fns) - pin
        return out

    bacc_mod.get_activation_tables = filtered


def _build_program():
    """Build + compile the SPMD Bass program (input-independent, cached)."""
    global _PROGRAM_CACHE
    if _PROGRAM_CACHE is not None:
        return _PROGRAM_CACHE
    _pin_act_table()
    _, _, mh_s, cnt, off = _made_struct()

    nc = bacc.Bacc("TRN2", target_bir_lowering=False, debug=False,
                   num_devices=NCORES)

    ctx_d = nc.dram_tensor("ctx", (BS, CTX), FP32, kind="ExternalInput")
    eps_d = nc.dram_tensor("eps", (S, BS, D), FP32, kind="ExternalInput")
    w1t_d = nc.dram_tensor("w1t", (D, HP), FP32, kind="ExternalInput")
    woutt_d = nc.dram_tensor("woutt", (32, D - 1, 2 * D), FP32,
                             kind="ExternalInput")
    wct_d = nc.dram_tensor("wct", (CTX, HP), FP32, kind="ExternalInput")
    b1_d = nc.dram_tensor("b1", (HP, 1), FP32, kind="ExternalInput")
    boutb_d = nc.dram_tensor("boutb", (128, 2 * D), FP32, kind="ExternalInput")
    ident_d = nc.dram_tensor("ident", (128, 128), FP32, kind="ExternalInput")
    z_d = nc.dram_tensor("z_out", (S, BS, D), FP32, kind="ExternalOutput")
    mu_d = nc.dram_tensor("mu_out", (S, BS, D), FP32, kind="ExternalOutput")
    sc_d = nc.dram_tensor("sc_out", (S, BS, D), FP32, kind="ExternalOutput")

    AF = mybir.ActivationFunctionType
    OP = mybir.AluOpType

    with tile.TileContext(nc) as tc, ExitStack() as ctx:
        singles = ctx.enter_context(tc.tile_pool(name="singles", bufs=1))
        ablk_pool = ctx.enter_context(tc.tile_pool(name="ablk", bufs=3))
        scratch = ctx.enter_context(tc.tile_pool(name="scratch", bufs=3))
        psA = ctx.enter_context(tc.tile_pool(name="psA", bufs=2, space="PSUM"))
        psOut = ctx.enter_context(tc.tile_pool(name="psOut", bufs=1,
                                               space="PSUM"))
        psZ = ctx.enter_context(tc.tile_pool(name="psZ", bufs=1, space="PSUM"))

        # ---- load inputs/constants into SBUF ----
        ctx_sb = singles.tile([BS, CTX], FP32)
        nc.sync.dma_start(ctx_sb[:], ctx_d.ap())
        w1t_sb = singles.tile([D, HP], FP32)
        nc.sync.dma_start(w1t_sb[:], w1t_d.ap())
        woutt_sb = singles.tile([32, D - 1, 2 * D], FP32)
        nc.sync.dma_start(woutt_sb[:], woutt_d.ap())
        wct_sb = singles.tile([128, 2, HP], FP32)
        nc.sync.dma_start(wct_sb[:],
                          wct_d.ap().rearrange("(k p) h -> p k h", p=128))
        b1_sb = singles.tile([128, HP // 128], FP32)
        nc.sync.dma_start(b1_sb[:],
                          b1_d.ap().rearrange("(c p) one -> p (c one)", p=128))
        boutb_sb = singles.tile([128, 2 * D], FP32)
        nc.sync.dma_start(boutb_sb[:], boutb_d.ap())
        boutbb_sb = singles.tile([1, 2 * D], BF16)
        nc.vector.tensor_copy(boutbb_sb[:], boutb_sb[0:1, :])
        ident_sb = singles.tile([128, 128], FP32)
        nc.sync.dma_start(ident_sb[:], ident_d.ap())
        ones_sb = singles.tile([1, 128], FP32)
        nc.vector.memset(ones_sb[:], 1.0)

        eps2 = [singles.tile([BS, 2, D], FP32, tag=f"eps{h}", name=f"eps{h}")
                for h in (0, 1)]
        for h in (0, 1):
            nc.sync.dma_start(
                eps2[h][:],
                eps_d.ap()[2 * h:2 * h + 2].rearrange("s b d -> b s d"))

        # ---- ctxT: (BS, CTX) -> (CTX, BS) in 2 chunks ----
        ctxT_sb = singles.tile([128, 2, BS], FP32)
        for k in range(2):
            ps = psA.tile([128, BS], FP32, tag="aps0")
            nc.tensor.transpose(ps[:], ctx_sb[:, k * 128:(k + 1) * 128],
                                ident_sb[:])
            nc.vector.tensor_copy(ctxT_sb[:, k, :], ps[:])

        # ---- A_base = WcT.T @ ctxT + b1 : (HP, BS) in 16 unit-chunks ----
        NCH = HP // 128
        a_base = singles.tile([128, NCH, BS], FP32)
        for hc in range(NCH):
            ps = psA.tile([128, BS], FP32, tag="aps0")
            for k in range(2):
                nc.tensor.matmul(
                    ps[:],
                    wct_sb[:, k, hc * 128:(hc + 1) * 128],
                    ctxT_sb[:, k, :],
                    start=(k == 0), stop=(k == 1))
            nc.vector.tensor_scalar_add(a_base[:, hc, :], ps[:],
                                        b1_sb[:, hc:hc + 1])

        # ---- per-half state ----
        z2 = [singles.tile([BS, 2, D], FP32, tag=f"z{h}", name=f"z{h}")
              for h in (0, 1)]
        mu2 = [singles.tile([BS, 2, D], FP32, tag=f"mu{h}", name=f"mu{h}")
               for h in (0, 1)]
        sc2 = [singles.tile([BS, 2, D], FP32, tag=f"sc{h}", name=f"sc{h}")
               for h in (0, 1)]
        zT2 = [singles.tile([D, RH], FP32, tag=f"zT{h}", name=f"zT{h}")
               for h in (0, 1)]
        outr2 = [psOut.tile([128, 2, 128], FP32, tag=f"outr{h}",
                              name=f"outr{h}") for h in (0, 1)]
        zTps2 = [psZ.tile([D, RH], FP32, tag=f"zTps{h}", name=f"zTps{h}")
                 for h in (0, 1)]

        for h in (0, 1):
            nc.vector.memset(z2[h][:], 0.0)

        def retranspose(i, h):
            """Re-transpose Z half h (cols > i garbage, rows > i of zT never
            read before refresh); copy row-group of row i psum->sbuf."""
            for s in (0, 1):
                nc.tensor.transpose(
                    zTps2[h][:, s * BS:(s + 1) * BS],
                    z2[h][:, s, :],
                    ident_sb[:])
            g = 32 * (i // 32)
            nc.vector.tensor_copy(zT2[h][g:g + 32, :], zTps2[h][g:g + 32, :])

        # ---- step 0 (bias-only): mu0 = bout[0], sc0 = softplus(bout[D]) ----
        for h in (0, 1):
            sp_tmp = scratch.tile([BS, 2], FP32, tag=f"sp{h}")
            nc.vector.tensor_copy(mu2[h][:, :, 0],
                                  boutb_sb[:, 0:1].to_broadcast((BS, 2)))
            nc.scalar.activation(out=sp_tmp[:],
                                 in_=boutb_sb[:, D:D + 1].to_broadcast((BS, 2)),
                                 func=AF.Exp, bias=0.0, scale=1.0)
            nc.scalar.activation(out=sc2[h][:, :, 0], in_=sp_tmp[:],
                                 func=AF.Ln, bias=1.0, scale=1.0)
            nc.vector.tensor_scalar(out=z2[h][:, :, 0], in0=eps2[h][:, :, 0],
                                    scalar1=sc2[h][:, 0, 0:1],
                                    scalar2=boutb_sb[:, 0:1],
                                    op0=OP.mult, op1=OP.add)
            retranspose(0, h)

        # ---- steps 1..63, two interleaved half-chains ----
        for i in range(1, D):
            nn = int(cnt[i])
            pp = 32 * (i - 1)              # padded unit offset of block i
            c, pl = pp // 128, pp % 128
            q = min(pl, 64)                # 32-aligned base (96 -> 64)
            kk = pl - q + nn
            for h in (0, 1):
                # fresh block pre-activation = ctx_base (shifted-identity MM)
                # + W1T[0:i, blk].T @ zT[0:i]
                aps_t = psA.tile([nn, RH], FP32, tag=f"aps{h}")
                sl = a_base[q:q + kk, c, :]
                rhs = bass.AP(sl.tensor, sl.offset,
                              [sl.ap[0], [0, 2], sl.ap[-1]])
                nc.tensor.matmul(aps_t[:],
                                 ident_sb[q:q + kk, pl:pl + nn],
                                 rhs, start=True, stop=False)
                nc.tensor.matmul(aps_t[:],
                                 w1t_sb[0:i, pp:pp + nn],
                                 zT2[h][0:i, :],
                                 start=False, stop=True)
                # relu -> bf16 (alternate engines across halves)
                ab = ablk_pool.tile([nn, RH], FP32, tag=f"ablk{h}")
                if h == 0:
                    nc.vector.tensor_scalar_max(ab[:], aps_t[:], 0.0)
                else:
                    nc.scalar.activation(out=ab[:], in_=aps_t[:],
                                         func=AF.Relu, bias=0.0, scale=1.0)

                if i == 1:
                    # seed OUT with bout once (ones ⊗ bout row)
                    for s in (0, 1):
                        nc.tensor.matmul(outr2[h][:, s, :], ones_sb[:],
                                         boutb_sb[0:1, :],
                                         start=(s == 0), stop=False,
                                         skip_group_check=True)

                # contribution: OUT[:, s, cols >= i] += ab_s.T @ WoutB[blk]
                # (cols < i are never read again -> halve the stream)
                for s in (0, 1):
                    for c0, c1 in ((i, D), (D + i, 2 * D)):
                        nc.tensor.matmul(outr2[h][:, s, c0:c1],
                                         ab[:, s * BS:(s + 1) * BS],
                                         woutt_sb[0:nn, i - 1, c0:c1],
                                         start=False,
                                         stop=(i == D - 1 and s == 1
                                               and c0 >= D),
                                         skip_group_check=True)

                # z-step: sc = softplus(OUT[ps]), z = OUT[mu] + sc*eps
                sp_tmp = scratch.tile([BS, 2], FP32, tag=f"sp{h}")
                nc.scalar.activation(out=sp_tmp[:], in_=outr2[h][:, :, D + i],
                                     func=AF.Exp, bias=0.0, scale=1.0)
                nc.scalar.activation(out=sc2[h][:, :, i], in_=sp_tmp[:],
                                     func=AF.Ln, bias=1.0, scale=1.0)
                tse = scratch.tile([BS, 2], FP32, tag=f"tse{h}")
                nc.vector.tensor_mul(tse[:], sc2[h][:, :, i], eps2[h][:, :, i])
                nc.vector.tensor_add(z2[h][:, :, i], tse[:],
                                     outr2[h][:, :, i])
                if i < D - 1:
                    retranspose(i, h)

        # ---- mu extraction (batched) + outputs ----
        for h in (0, 1):
            nc.vector.tensor_copy(mu2[h][:, :, 1:D], outr2[h][:, :, 1:D])
            nc.sync.dma_start(
                z_d.ap()[2 * h:2 * h + 2].rearrange("s b d -> b s d"),
                z2[h][:])
            nc.sync.dma_start(
                mu_d.ap()[2 * h:2 * h + 2].rearrange("s b d -> b s d"),
                mu2[h][:])
            nc.sync.dma_start(
                sc_d.ap()[2 * h:2 * h + 2].rearrange("s b d -> b s d"),
                sc2[h][:])

    nc.compile()
    _PROGRAM_CACHE = nc
    return nc


def _in_maps(context, eps, W1, b1, Wc, Wout, bout):
    W1T, WoutB, WcT, b1p = _prep_weights(W1, b1, Wc, Wout)
    ident = np.eye(128, dtype=np.float32)
    boutb = np.ascontiguousarray(np.tile(bout.reshape(1, -1), (128, 1)))
    maps = []
    for c in range(NCORES):
        maps.append({
            "ctx": np.ascontiguousarray(context[c * BS:(c + 1) * BS]),
            "eps": np.ascontiguousarray(eps[:, c * BS:(c + 1) * BS]),
            "w1t": W1T, "woutt": WoutB, "wct": WcT, "b1": b1p,
            "boutb": boutb, "ident": ident,
        })
    return maps


def run(context, eps, W1, b1, Wc, Wout, bout, trace=False):
    context = np.asarray(context, np.float32)
    eps = np.asarray(eps, np.float32)
    W1 = np.asarray(W1, np.float32)
    b1 = np.asarray(b1, np.float32)
    Wc = np.asarray(Wc, np.float32)
    Wout = np.asarray(Wout, np.float32)
    bout = np.asarray(bout, np.float32)
    nc = _build_program()
    maps = _in_maps(context, eps, W1, b1, Wc, Wout, bout)
    res = run_bass_kernel_spmd(nc, maps, core_ids=list(range(NCORES)),
                               trace=trace)
    z = np.empty((S, B, D), np.float32)
    mu = np.empty((S, B, D), np.float32)
    sc = np.empty((S, B, D), np.float32)
    for c in range(NCORES):
        z[:, c * BS:(c + 1) * BS] = res.results[c]["z_out"]
        mu[:, c * BS:(c + 1) * BS] = res.results[c]["mu_out"]
        sc[:, c * BS:(c + 1) * BS] = res.results[c]["sc_out"]
    return (z, mu, sc), res


def kernel(context, eps, W1, b1, Wc, Wout, bout):
    (z, mu, sc), _ = run(context, eps, W1, b1, Wc, Wout, bout)
    return z, mu, sc



# revision 20
# speedup vs baseline: 1.3000x; 1.3000x over previous
"""Trainium2 Bass kernel for nn_AutoRegressiveDistribution (MADE sampling).

Self-contained: hardcodes shapes/sharding. Shards batch B across 8 cores,
runs the D-step autoregressive sampling loop fully on-device per core.

Per-core structure (v2): FOUR independent chains, one per sample s
(width = BS = 128 batch rows each). Per chain and step i the critical
path is:

  hist-MM (PE, float32r, N=256 via free-dim duplication -> 1 cyc/row)
    -> relu (DVE, psum->sbuf, bf16-free)
    -> pair-MM (PE, N=2: only the (mu_i, ps_i) column pair, Wout columns
       are pair-interleaved host-side so the pair is contiguous)
    -> Exp -> Ln(bias=1) (Act, psum->psum softplus)
    -> fused FMA z = eps*sc + mu (GPSIMD scalar_tensor_tensor)
    -> full-state transpose (PE) -> single-row psum->sbuf copy (GPSIMD)
    -> next hist-MM

  The bulk contribution of block i to future column pairs [2i+2, 128)
  is emitted AFTER the FMA so it stays off the critical path; the
  ctx+bias base for the next block is preloaded into PSUM by a
  shifted-identity matmul (also f32r/dup). The loop-invariant
  a_base = Wc @ ctx + b1 is computed chunk-by-chunk interleaved with
  the first ~16 steps so it never stalls the chain.

  float32r bitcasts keep full fp32 precision at 1 cycle/row (vs 4 for
  fp32) for every z-path matmul; only PE-stationary operands and the
  tiny N<=2 pair matmuls stay plain fp32.
"""

import numpy as np
from contextlib import ExitStack

import concourse.bass as bass
import concourse.tile as tile
from concourse import bacc, mybir
from concourse.bass_utils import run_bass_kernel_spmd

D, H, CTX, B, S = 64, 1024, 256, 1024, 4
NCORES = 8
BS = B // NCORES          # 128 batch rows per core
K = S                     # 4 chains per core, one per sample
HP = 2048                 # padded hidden units: block i at [32*(i-1), +cnt[i])
NCH = HP // 128           # a_base unit chunks

FP32 = mybir.dt.float32
F32R = mybir.dt.float32r


def _made_struct():
    mh = (np.arange(H) % (D - 1)) + 1            # degrees 1..63
    perm = np.argsort(mh, kind="stable")
    mh_s = mh[perm]
    cnt = np.bincount(mh_s, minlength=D)          # cnt[d] = #units of degree d
    off = np.concatenate([[0], np.cumsum(cnt)[:-1]]).astype(np.int64)
    return mh, perm, mh_s, cnt, off


def _prep_weights(W1, b1, Wc, Wout):
    """Mask + permute + 32-pad weights host-side (cheap, O(weight size))."""
    mh, perm, mh_s, cnt, off = _made_struct()
    m0 = np.arange(1, D + 1)
    M1 = (mh[:, None] >= m0[None, :]).astype(np.float32)          # (H, D)
    mout = np.concatenate([m0, m0])                                # (2D,)
    Mout = (mout[:, None] > mh[None, :]).astype(np.float32)        # (2D, H)
    W1m = (W1 * M1)[perm]                   # (H, D) permuted rows
    Woutm = (Wout * Mout)[:, perm]          # (2D, H) permuted cols
    src = np.arange(H)
    pdst = 32 * (mh_s - 1) + (src - off[mh_s])   # padded slot of sorted unit
    W1T = np.zeros((D, HP), np.float32)
    W1T[:, pdst] = W1m.T
    WcT = np.zeros((CTX, HP), np.float32)
    WcT[:, pdst] = Wc[perm].T
    b1c = np.zeros((128, NCH), np.float32)
    b1p = np.zeros((HP,), np.float32)
    b1p[pdst] = b1[perm]
    b1c[:, :] = b1p.reshape(NCH, 128).T
    # pair-interleaved output weights: col 2j = mu_j, col 2j+1 = prescale_j
    WoutP = np.zeros((32, D - 1, 2 * D), np.float32)
    mu_rows = Woutm[:D, :]      # (D, H)
    ps_rows = Woutm[D:, :]      # (D, H)
    for j in range(D):
        WoutP[pdst % 32, (mh_s - 1), 2 * j] = mu_rows[j, src]
        WoutP[pdst % 32, (mh_s - 1), 2 * j + 1] = ps_rows[j, src]
    return W1T, WoutP, WcT, b1c


def _prep_bout(bout):
    boutP = np.zeros((128, 2 * D), np.float32)
    boutP[:, 0::2] = bout[:D][None, :]
    boutP[:, 1::2] = bout[D:][None, :]
    return boutP


_PROGRAM_CACHE = None


def _pin_act_table():
    """Make Exp/Ln/Relu resolvable only via natural_log_exp_and_others so
    the act-table chooser doesn't thrash (each LoadActFuncSet ~1.3us)."""
    import concourse.bacc as bacc_mod
    from concourse import hw_specs
    orig = hw_specs.get_activation_tables
    AF = mybir.ActivationFunctionType
    pin = {AF.Exp, AF.Ln, AF.Relu}

    def filtered(arch):
        out = {}
        for name, fns in orig(arch).items():
            if name == "natural_log_exp_and_others":
                out[name] = set(fns)
            else:
                out[name] = set(fns) - pin
        return out

    bacc_mod.get_activation_tables = filtered


def _dup(ap):
    """Duplicate an AP along a broadcast free dim (doubles free size so
    float32r matmuls hit N>=256 and run at 1 cycle/row)."""
    return bass.AP(ap.tensor, ap.offset, [ap.ap[0], [0, 2], ap.ap[-1]])


def _build_program():
    global _PROGRAM_CACHE
    if _PROGRAM_CACHE is not None:
        return _PROGRAM_CACHE
    _pin_act_table()
    _, _, mh_s, cnt, off = _made_struct()

    nc = bacc.Bacc("TRN2", target_bir_lowering=False, debug=False,
                   num_devices=NCORES)

    ctx_d = nc.dram_tensor("ctx", (BS, CTX), FP32, kind="ExternalInput")
    eps_d = nc.dram_tensor("eps", (S, BS, D), FP32, kind="ExternalInput")
    w1t_d = nc.dram_tensor("w1t", (D, HP), F32R, kind="ExternalInput")
    woutp_d = nc.dram_tensor("woutp", (32, D - 1, 2 * D), FP32,
                             kind="ExternalInput")
    wct_d = nc.dram_tensor("wct", (CTX, HP), F32R, kind="ExternalInput")
    b1c_d = nc.dram_tensor("b1c", (128, NCH), FP32, kind="ExternalInput")
    boutp_d = nc.dram_tensor("boutp", (128, 2 * D), FP32, kind="ExternalInput")
    ident_d = nc.dram_tensor("ident", (128, 128), F32R, kind="ExternalInput")
    z_d = nc.dram_tensor("z_out", (S, BS, D), FP32, kind="ExternalOutput")
    mu_d = nc.dram_tensor("mu_out", (S, BS, D), FP32, kind="ExternalOutput")
    sc_d = nc.dram_tensor("sc_out", (S, BS, D), FP32, kind="ExternalOutput")

    AF = mybir.ActivationFunctionType
    OP = mybir.AluOpType

    with tile.TileContext(nc) as tc, ExitStack() as ctx:
        singles = ctx.enter_context(tc.tile_pool(name="singles", bufs=1))
        abp = ctx.enter_context(tc.tile_pool(name="abp", bufs=2))
        psum = ctx.enter_context(tc.tile_pool(name="psum", bufs=1,
                                              space="PSUM"))

        # ---- input DMAs, priority order ----
        ctx_sb = singles.tile([BS, CTX], FP32)
        nc.sync.dma_start(ctx_sb[:], ctx_d.ap())
        eps_sb = singles.tile([BS, S, D], FP32)
        nc.sync.dma_start(eps_sb[:], eps_d.ap().rearrange("s b d -> b s d"))
        boutp_sb = singles.tile([128, 2 * D], FP32)
        nc.sync.dma_start(boutp_sb[:], boutp_d.ap())
        ident_sb = singles.tile([128, 128], F32R)
        nc.sync.dma_start(ident_sb[:], ident_d.ap())
        wct_sb = singles.tile([128, 2, HP], F32R)
        w1t_sb = singles.tile([D, HP], F32R)
        woutp_sb = singles.tile([32, D - 1, 2 * D], FP32)
        QH = HP // 4
        for q in range(4):
            nc.sync.dma_start(
                wct_sb[:, :, q * QH:(q + 1) * QH],
                wct_d.ap()[:, q * QH:(q + 1) * QH]
                .rearrange("(k p) h -> p k h", p=128))
            nc.sync.dma_start(w1t_sb[:, q * QH:(q + 1) * QH],
                              w1t_d.ap()[:, q * QH:(q + 1) * QH])
            b0, b1_ = [(0, 16), (16, 32), (32, 48), (48, 63)][q]
            nc.sync.dma_start(woutp_sb[:, b0:b1_, :],
                              woutp_d.ap()[:, b0:b1_, :])
        b1c_sb = singles.tile([128, NCH], FP32)
        nc.sync.dma_start(b1c_sb[:], b1c_d.ap())

        ones_sb = singles.tile([1, 128], FP32)
        nc.vector.memset(ones_sb[:], 1.0)

        # ---- PSUM layout, shaped by the HW rule that a start=True matmul
        # marks its whole 2KB bank pending-zero (so a bank can host only one
        # accumulation lifetime at a time):
        #  bank tOUT: all 4 persistent OUT accumulators, seeded by ONE
        #             start=True matmul, then only start=False forever.
        #  bank tZT:  all 4 transpose targets (every write is a fresh
        #             single-matmul start=True group -> safe to share).
        #  banks tPA[c]: per-chain psA (ident start=True -> hist stop,
        #             WAW-ordered, nothing else matmuls this bank) plus the
        #             Act-written scPS strip (engines ignore pending flags).
        #  bank tSET: setup scratch; only single-matmul start=True groups.
        tOUT = psum.tile([128, K, 2 * D], FP32, name="tOUT")
        tZT = psum.tile([D, K, BS], FP32, name="tZT")
        tPA = [psum.tile([128, 512], FP32, tag=f"tPA{c}", name=f"tPA{c}")
               for c in range(K)]
        tSET = psum.tile([128, 512], FP32, name="tSET")
        outP = [tOUT[:, c, :] for c in range(K)]              # (128, 128)
        zTps = [tZT[:, c, :] for c in range(K)]               # (64, 128)
        SC0 = 256                                             # scPS base col

        def aps_ap(c, nn):
            """psA view of tPA[c]: (nn, 2, 128) at cols 0:256."""
            t = tPA[c][0:nn, 0:256]
            return bass.AP(t.tensor, t.offset, [t.ap[0], [128, 2], [1, 128]])

        # ---- ctxT: (BS, CTX) -> (128, 2, BS) via 2 PE transposes ----
        ctxT_sb = singles.tile([128, 2, BS], F32R)
        for kk in range(2):
            ps = tSET[:, kk * BS:kk * BS + BS]
            nc.tensor.transpose(ps, ctx_sb[:, kk * 128:(kk + 1) * 128],
                                ident_sb[:].bitcast(FP32))
            nc.vector.tensor_copy(ctxT_sb[:, kk, :], ps)

        # ---- per-chain state ----
        a_base = singles.tile([128, NCH, 128], F32R)
        z2 = [singles.tile([BS, D], FP32, tag=f"z{c}", name=f"z{c}")
              for c in range(K)]
        muA = singles.tile([BS, K, D], FP32)
        scA = singles.tile([BS, K, D], FP32)
        zT = [singles.tile([D, BS], F32R, tag=f"zT{c}", name=f"zT{c}")
              for c in range(K)]

        for c in range(K):
            nc.vector.memset(z2[c][:], 0.0)

        def a_base_chunk(cc):
            """a_base[:, cc, :] = (WcT chunk).T @ ctxT + b1 chunk.

            In-place accumulation in the tSET bank is safe: every matmul
            writer of this bank is ordered by WAW or data deps, so no
            start=True interloper can land between the two halves."""
            ps = tSET[:, 256:384]
            for kk in range(2):
                nc.tensor.matmul(
                    ps,
                    wct_sb[:, kk, cc * 128:(cc + 1) * 128],
                    ctxT_sb[:, kk, :],
                    start=(kk == 0), stop=(kk == 1))
            nc.vector.tensor_scalar_add(a_base[:, cc, :], ps,
                                        b1c_sb[:, cc:cc + 1])

        a_base_chunk(0)
        a_base_chunk(1)

        def bridge(c, i):
            """z2[c] -> zT[c] row i: full-state PE transpose + 1-row copy."""
            nc.tensor.transpose(zTps[c], z2[c][:], ident_sb[:].bitcast(FP32))
            g = 32 * (i // 32)
            src_rows = tZT[g:g + 32, c, :]
            if c < 2:
                nc.vector.tensor_copy(zT[c][g:g + 32, :], src_rows)
            else:
                nc.scalar.copy(zT[c][g:g + 32, :], src_rows)

        # ---- step 0: bias-only ----
        # one seed matmul covers all 4 OUT accumulators (single start=True
        # lifetime for the whole bank)
        br = boutp_sb[0:1, :]
        br4 = bass.AP(br.tensor, br.offset, [br.ap[0], [0, K], br.ap[-1]])
        nc.tensor.matmul(tOUT[:, :, :], ones_sb[:], br4,
                         start=True, stop=False, skip_group_check=True)
        for c in range(K):
            sp = tPA[c][:, SC0 + D:SC0 + D + 1]
            nc.scalar.activation(out=sp, in_=boutp_sb[:, 1:2],
                                 func=AF.Exp, bias=0.0, scale=1.0)
            nc.scalar.activation(out=tPA[c][:, SC0:SC0 + 1], in_=sp,
                                 func=AF.Ln, bias=1.0, scale=1.0)
            nc.vector.scalar_tensor_tensor(
                out=z2[c][:, 0:1], in0=eps_sb[:, c, 0:1],
                scalar=tPA[c][:, SC0:SC0 + 1], in1=boutp_sb[:, 0:1],
                op0=OP.mult, op1=OP.add)
            bridge(c, 0)

        # ---- steps 1..63 ----
        for i in range(1, D):
            nn = int(cnt[i])
            pp = 32 * (i - 1)
            cc, pl = pp // 128, pp % 128
            kk = pl + nn                  # ident rows anchored at 0 so both
                                          # group matmuls share tile pos (0,0)
            if i >= 5 and (i - 5) % 4 == 0:
                nxt = 2 + (i - 5) // 4
                if nxt < NCH:
                    a_base_chunk(nxt)
            for c in range(K):
                # base preload (shifted identity) + history matmul
                aps = aps_ap(c, nn)
                nc.tensor.matmul(
                    aps,
                    ident_sb[0:kk, pl:pl + nn],
                    _dup(a_base[0:kk, cc, :]),
                    start=True, stop=False)
                nc.tensor.matmul(
                    aps,
                    w1t_sb[0:i, pp:pp + nn],
                    _dup(zT[c][0:i, :]),
                    start=False, stop=True)
                # relu -> ab (sbuf)
                ab = abp.tile([nn, 128], FP32, tag=f"ab{c}")
                nc.vector.tensor_scalar_max(ab[:], tPA[c][0:nn, 0:128], 0.0)
                # immediate pair columns (unblocks softplus asap)
                nc.tensor.matmul(tOUT[:, c, 2 * i:2 * i + 2],
                                 ab[:], woutp_sb[0:nn, i - 1,
                                                 2 * i:2 * i + 2],
                                 start=False, stop=(i == D - 1),
                                 skip_group_check=True)
                # softplus: Exp -> Ln(1+x), psum->psum
                sp = tPA[c][:, SC0 + D + (i % 2):SC0 + D + (i % 2) + 1]
                nc.scalar.activation(out=sp,
                                     in_=tOUT[:, c, 2 * i + 1:2 * i + 2],
                                     func=AF.Exp, bias=0.0, scale=1.0)
                nc.scalar.activation(out=tPA[c][:, SC0 + i:SC0 + i + 1],
                                     in_=sp, func=AF.Ln, bias=1.0, scale=1.0)
                # fused z = eps*sc + mu
                nc.vector.scalar_tensor_tensor(
                    out=z2[c][:, i:i + 1], in0=eps_sb[:, c, i:i + 1],
                    scalar=tPA[c][:, SC0 + i:SC0 + i + 1],
                    in1=tOUT[:, c, 2 * i:2 * i + 1],
                    op0=OP.mult, op1=OP.add)
                if i < D - 1:
                    bridge(c, i)
                    # deferred bulk contribution to future pairs (off-chain)
                    nc.tensor.matmul(tOUT[:, c, 2 * i + 2:2 * D],
                                     ab[:], woutp_sb[0:nn, i - 1, 2 * i + 2:],
                                     start=False, stop=False,
                                     skip_group_check=True)

        # ---- tail: extract mu (even cols) + sc, DMA out ----
        for c in range(K):
            sl = tOUT[:, c, :]
            mu_ap = bass.AP(sl.tensor, sl.offset, [sl.ap[0], [2, D]])
            nc.vector.tensor_copy(muA[:, c, :], mu_ap)
            nc.scalar.copy(scA[:, c, :], tPA[c][:, SC0:SC0 + D])
            nc.sync.dma_start(z_d.ap()[c], z2[c][:])
        nc.sync.dma_start(mu_d.ap().rearrange("s b d -> b s d"), muA[:])
        nc.sync.dma_start(sc_d.ap().rearrange("s b d -> b s d"), scA[:])

    nc.compile()
    _PROGRAM_CACHE = nc
    return nc


def _in_maps(context, eps, W1, b1, Wc, Wout, bout):
    W1T, WoutP, WcT, b1c = _prep_weights(W1, b1, Wc, Wout)
    boutP = _prep_bout(bout)
    ident = np.eye(128, dtype=np.float32)
    maps = []
    for c in range(NCORES):
        maps.append({
            "ctx": np.ascontiguousarray(context[c * BS:(c + 1) * BS]),
            "eps": np.ascontiguousarray(eps[:, c * BS:(c + 1) * BS]),
            "w1t": W1T, "woutp": WoutP, "wct": WcT, "b1c": b1c,
            "boutp": boutP, "ident": ident,
        })
    return maps


def run(context, eps, W1, b1, Wc, Wout, bout, trace=False):
    context = np.asarray(context, np.float32)
    eps = np.asarray(eps, np.float32)
    W1 = np.asarray(W1, np.float32)
    b1 = np.asarray(b1, np.float32)
    Wc = np.asarray(Wc, np.float32)
    Wout = np.asarray(Wout, np.float32)
    bout = np.asarray(bout, np.float32)
    nc = _build_program()
    maps = _in_maps(context, eps, W1, b1, Wc, Wout, bout)
    res = run_bass_kernel_spmd(nc, maps, core_ids=list(range(NCORES)),
                               trace=trace)
    z = np.empty((S, B, D), np.float32)
    mu = np.empty((S, B, D), np.float32)
    sc = np.empty((S, B, D), np.float32)
    for c in range(NCORES):
        z[:, c * BS:(c + 1) * BS] = res.results[c]["z_out"]
        mu[:, c * BS:(c + 1) * BS] = res.results[c]["mu_out"]
        sc[:, c * BS:(c + 1) * BS] = res.results[c]["sc_out"]
    return (z, mu, sc), res


def kernel(context, eps, W1, b1, Wc, Wout, bout):
    (z, mu, sc), _ = run(context, eps, W1, b1, Wc, Wout, bout)
    return z, mu, sc
